# revision 41
# baseline (speedup 1.0000x reference)
"""CPC loss kernel for Trainium2 (8 NeuronCores, SPMD data-parallel over batch N).

Math (per batch element n, handled by core n):
  Az[t]   = W @ latent[n, t]            (K*C = 3072 outputs per position)
  scores[t, k, m] = phi[s_{t,m}] . Az[t, k]   (M=128 negatives per position)
  num[t, k]       = latent[n, 1+t+k] . Az[t, k]
  loss = mean over (n, t<500, k) of log(sum_m exp(scores) + exp(num)) - num

Device strategy per core (fp8 streaming, no on-device gather):
  - The negative-sample gather is a pure data rearrangement with indices known
    at kernel-build time, so the host materializes the per-position rhs stream
    in fp8-e4m3 directly in the [c, cols]-layout the PE needs: for each
    (position t, c-half h) a 140-col block = 12 positive latent cols
    (latent[1+t+j], diagonal j==k extracted later) ++ 128 gathered negatives.
    The device streams it with plain wide DMAs (16 chunks, ~1.15 MB each,
    ~18.4 MB total) -- no SWDGE descriptor generation at all.
  - AzT is computed once via 24 fp8 DoubleRow matmuls (one per kc-tile, both
    c-halves contracted per instruction) into 2-bank PSUM tiles and stored fp8
    in SBUF as azk[c_half, k, h, t] with one contiguous 1 KB/partition copy
    per k (alternating ACT/DVE); score-matmul lhsT slabs are strided column
    views of it.
  - Per position one matmul pair (h-halves accumulating) with lhsT
    azk[:, :, h, t] computes positives and negatives together: [32, 140] into
    the half-megatile's PSUM bank at tile_position (0, 32q).  Per 8-position
    half-megatile (2 banks, 4-deep pipeline) one ACT exp(x-50) (bf16 out) +
    DVE reduce yields sum_m exp(scores-50) and one batched DVE mul+reduce
    extracts num; exp(num-50) is folded into tot once at the end.
  - Final: ln(tot*2^-32) with accum_out row-sum, minus the num row-sum,
    masked partition-sum via 1-col matmul.
Host: loss = sum(partials)/48000 + 50 + 32*ln(2).
"""

import sys, os

_ABL = ""

for _p in ("/opt/trn_rl_repo", "/root/.axon_site/_ro/trn_rl_repo"):
    if _p not in sys.path:
        sys.path.append(_p)

import numpy as np
import ml_dtypes

import concourse.bass as bass
import concourse.bacc as bacc
import concourse.mybir as mybir
from concourse.tile import TileContext, add_dep_helper

BF16 = ml_dtypes.bfloat16
FP8 = ml_dtypes.float8_e4m3

N, T, C, K, M = 8, 512, 256, 12, 128
Tp = T - K  # 500 real positions
TPAD = 512  # padded position count (64 PSUM half-megatiles of 8)
PB = 12  # positive cols per position (k = 0..11)
FB = PB + M  # 140 stream cols per (position, c-half)
SHIFT = 50.0  # fixed logsumexp shift; |scores| << SHIFT + 88 so exp never overflows
DENOM = N * Tp * K  # 48000
NCHUNK = 32  # phi streaming chunks (16 positions each)
CPOS = TPAD // NCHUNK  # positions per chunk


def build_bass():
    nc = bacc.Bacc(
        "TRN2",
        target_bir_lowering=False,
        debug=False,
        enable_asserts=False,
    )
    dt = mybir.dt
    DR = mybir.MatmulPerfMode.DoubleRow

    # phi8[p, t, h, 0:12]   = fp8(latent[n, 1+t+j, h*128 + p]) (positives)
    # phi8[p, t, h, 12:140] = fp8(latent[samps[n,t,m] // T, _ % T, h*128 + p])
    phi8 = nc.dram_tensor("phi8", [128, TPAD * 2 * FB], dt.float8e4, kind="ExternalInput").ap()
    latT8 = nc.dram_tensor("latT8", [128, 2, T], dt.float8e4, kind="ExternalInput").ap()
    wT8 = nc.dram_tensor("wT8", [128, 2 * K, 2, 128], dt.float8e4, kind="ExternalInput").ap()
    pmask = nc.dram_tensor("pmask", [128, 1], dt.float32, kind="ExternalInput").ap()
    maskI = nc.dram_tensor("maskI", [128, 2 * PB], dt.float32, kind="ExternalInput").ap()
    out = nc.dram_tensor("out", [1, 1], dt.float32, kind="ExternalOutput").ap()

    with TileContext(nc) as tc:
        with (
            tc.tile_pool(name="const", bufs=1) as cp,
            tc.tile_pool(name="phi", bufs=1) as pp,
            tc.tile_pool(name="scr", bufs=8) as sp,
            tc.tile_pool(name="acc", bufs=1) as ap_,
        ):
            # --- constant / weight loads -------------------------------------
            latT8_t = cp.tile([128, 2, T], dt.float8e4)
            nc.sync.dma_start(latT8_t[:], latT8[:])
            # per-kc-tile-major W layout, split into two DMAs so the first Az
            # matmuls can start as soon as the first half lands
            wT8_t = cp.tile([128, 2 * K, 2, 128], dt.float8e4)
            nc.sync.dma_start(wT8_t[:, :K], wT8[:, :K])
            nc.sync.dma_start(wT8_t[:, K:], wT8[:, K:])
            negshift = cp.tile([128, 1], dt.float32)
            nc.vector.memset(negshift[:], -SHIFT)
            # preload both ACT table sets (exp, ln) while the DMAs run so the
            # first real exp / final ln don't stall on a ~1.3us table load
            tldca = cp.tile([128, 1], dt.float32)
            tldcb = cp.tile([128, 1], dt.float32)
            nc.vector.memset(tldca[:], 1.0)
            nc.scalar.activation(out=tldcb[:], in_=tldca[:], func=mybir.ActivationFunctionType.Exp)
            nc.scalar.activation(out=tldcb[:], in_=tldca[:], func=mybir.ActivationFunctionType.Ln)

            pmask_t = cp.tile([128, 1], dt.float32)
            nc.sync.dma_start(pmask_t[:], pmask[:])
            maskI_t = cp.tile([128, 2, PB], dt.float32)
            nc.sync.dma_start(maskI_t[:], maskI[:].rearrange("p (s j) -> p s j", j=PB))

            # AzT store: azk[p, k, h, t] = Az[k, h*128+p, t] (fp8), k padded
            # to 32 with zeros so the pad output partitions produce zero
            # scores.  (h, t) innermost so each k's PSUM->SBUF copy is one
            # contiguous 1 KB/partition move; lhsT slabs take strided columns.
            azsb = ap_.tile([128, 32 * 2 * T], dt.float8e4)
            azk = azsb.rearrange("p (k hh t) -> p k hh t", hh=2, k=32)
            # pad k-cols zeroed on DVE: it is idle during the PE warmup and
            # the first Az matmuls, and keeping this fill off the DMA queue
            # preserves the phi chunks' arrival margin over their consumers
            azpad = azk[:, K:32, :, :]
            nc.vector.memzero(azpad[:, : (32 - K) // 2])
            nc.vector.memzero(azpad[:, (32 - K) // 2 :])

            # --- phi stream: 32 chunks of 16 positions (~4.5 KB/partition)
            phi_t = pp.tile([128, TPAD * 2 * FB], dt.float8e4)
            phi4 = phi_t.rearrange("p (t hh j) -> p t hh j", hh=2, j=FB)
            for ch in range(NCHUNK):
                c0 = ch * CPOS * 2 * FB
                c1 = (ch + 1) * CPOS * 2 * FB
                nc.sync.dma_start(phi_t[:, c0:c1], phi8[:, c0:c1])

            # tot in bf16: 2-byte src+dst lets the DVE reduce run in 2x_1P
            # packed mode (validated: < 1e-5 impact on the loss)
            tot_all = ap_.tile([128, TPAD // 4], dt.bfloat16)
            num_all = ap_.tile([128, TPAD // 4], dt.float32)

            # --- Az phase: AzT[kc, t] = sum_c' W[kc, c'] latent[n, t, c'] ----
            # One DoubleRow matmul per kc-tile contracts both c'-halves; both
            # h-halves of a k land in one 2-bank PSUM tile so a single
            # contiguous copy (alternating ACT/DVE) moves them to SBUF.
            # The Az and score phases share one PSUM pool (same tile shape) so
            # there is no pool-transition barrier between them.
            wtile = cp.tile([128, 256], dt.bfloat16)
            nc.vector.memset(wtile[:], 0.5)
            with tc.tile_pool(name="ps", bufs=4, space="PSUM") as scps:
                # dummy matmuls ramp the PE p-state out of its cold clock
                # while the weight DMAs are still in flight, so the real Az
                # matmuls don't run 6x slow
                W0 = scps.tile([128, 2, T], dt.float32, name="P")
                for _ in range(12):
                    nc.tensor.matmul(
                        W0[:, 0, 0:128],
                        lhsT=wtile[:, 0:128],
                        rhs=wtile[:, 128:256],
                        start=True,
                        stop=True,
                    )
                for k_ in range(K):
                    pa = scps.tile([128, 2, T], dt.float32, name="P")
                    for h_ in range(2):
                        nc.tensor.matmul(
                            pa[:, h_, :],
                            lhsT=wT8_t[:, 2 * k_ + h_, :, :],
                            rhs=latT8_t[:, :, :],
                            start=True,
                            stop=True,
                            perf_mode=DR,
                        )
                    if k_ % 2 == 0:
                        nc.scalar.copy(out=azk[:, k_, :, :], in_=pa[:, :, :])
                    else:
                        nc.vector.tensor_copy(out=azk[:, k_, :, :], in_=pa[:, :, :])

                # --- score half-megatiles ------------------------------------
                _nmega = TPAD // 8
                for mega in range(_nmega):
                    P = scps.tile([128, 2, 512], dt.float32, name="P")
                    if "nomm" not in _ABL:
                        for s in range(2):  # bank = one 4-position score tile
                            for q in range(4):
                                t = (mega * 2 + s) * 4 + q
                                for h in range(2):
                                    nc.tensor.matmul(
                                        P[32 * q : 32 * q + 32, s, 0:FB],
                                        lhsT=azk[:, :, h, t],
                                        rhs=phi4[:, t, h, :],
                                        start=(h == 0),
                                        stop=(h == 1),
                                        tile_position=(0, 32 * q),
                                    )
                    if "notail" in _ABL:
                        continue
                    # tot[t,k] = sum_m exp(score-50): one exp over both banks
                    E4 = sp.tile([128, 2, M], dt.bfloat16, tag="exp", name="exp_o")
                    nc.scalar.activation(
                        out=E4[:],
                        in_=P[:, :, PB : PB + M],
                        func=mybir.ActivationFunctionType.Exp,
                        bias=negshift[:],
                        scale=1.0,
                    )
                    # num[t,k] -> num_all (diagonal j==k of the pos blocks),
                    # both banks in one batched mul+reduce.  The mul goes on
                    # the DVE queue BEFORE the exp-dependent tot-reduce: the
                    # PSUM buffer is released by {exp, mul}, so this keeps the
                    # release off the exp->reduce semaphore chain.
                    scr = sp.tile([128, 2, PB], dt.float32, tag="ttr", name="ttr_o")
                    nc.vector.tensor_mul(scr[:], P[:, :, 0:PB], maskI_t[:])
                    nc.vector.tensor_reduce(
                        num_all[:, mega * 2 : mega * 2 + 2],
                        scr[:],
                        axis=mybir.AxisListType.X,
                        op=mybir.AluOpType.add,
                    )
                    with nc.allow_low_precision(reason="bf16 tot validated <1e-5"):
                        nc.vector.tensor_reduce(
                            tot_all[:, mega * 2 : mega * 2 + 2],
                            E4[:],
                            axis=mybir.AxisListType.X,
                            op=mybir.AluOpType.add,
                        )

            # --- final reduction --------------------------------------------
            if "nofin" in _ABL:
                dummy = ap_.tile([1, 1], dt.float32)
                nc.vector.memset(dummy[:], 0.0)
                nc.sync.dma_start(out[:], dummy[:])
            else:
                NV = Tp // 4  # 125 valid score tiles
                # row-sum of num (DVE) runs concurrently with exp(num-50) (ACT)
                numsum = ap_.tile([128, 1], dt.float32)
                nc.vector.tensor_reduce(
                    numsum[:],
                    num_all[:, :NV],
                    axis=mybir.AxisListType.X,
                    op=mybir.AluOpType.add,
                )
                # fold in the positive term for all valid tiles at once:
                # tot += exp(num - 50)
                en_t = ap_.tile([128, NV], dt.bfloat16)
                nc.scalar.activation(
                    out=en_t[:],
                    in_=num_all[:, :NV],
                    func=mybir.ActivationFunctionType.Exp,
                    bias=negshift[:],
                    scale=1.0,
                )
                nc.vector.tensor_add(
                    tot_all[:, :NV], tot_all[:, :NV], en_t[:]
                )
                # ln(tot * 2^-32) keeps the ACT-ln input within its 2^64 valid
                # range for extreme scores; +32*ln2 is restored on the host.
                # accum_out row-sums the ln values in the same instruction.
                Lt = ap_.tile([128, NV], dt.float32)
                lnsum = ap_.tile([128, 1], dt.float32)
                nc.scalar.activation(
                    out=Lt[:],
                    in_=tot_all[:, :NV],
                    func=mybir.ActivationFunctionType.Ln,
                    scale=float(2.0**-32),
                    accum_out=lnsum[:],
                )
                rs = ap_.tile([128, 1], dt.float32)
                nc.vector.tensor_sub(rs[:], lnsum[:], numsum[:])
                with tc.tile_pool(name="f_ps", bufs=1, space="PSUM") as fps:
                    psf = fps.tile([1, 1], dt.float32)
                    nc.tensor.matmul(psf[:], lhsT=rs[:], rhs=pmask_t[:])
                    outsb = ap_.tile([1, 1], dt.float32)
                    nc.scalar.copy(out=outsb[:], in_=psf[:])
                    nc.sync.dma_start(out[:], outsb[:])

    nc.compile()
    return nc


def prep_inputs(latent, W, samps):
    """Host-side sharding + layout marshalling. Returns per-core input maps."""
    latent = np.asarray(latent, dtype=np.float32)
    W = np.asarray(W, dtype=np.float32)
    samps = np.asarray(samps).astype(np.int64).reshape(N, Tp, M)

    lat8_all = latent.reshape(N * T, C).astype(FP8)
    # wT8[p, b, h, j] = W[b*128 + j, h*128 + p]
    wT8 = np.ascontiguousarray(
        W.astype(FP8).reshape(2 * K, 128, 2, 128).transpose(3, 0, 2, 1)
    )
    pmask = ((np.arange(128) % 32) < K).astype(np.float32).reshape(128, 1)
    k_arr = np.arange(128) % 32
    maskD = (
        (np.arange(PB)[None, :] == k_arr[:, None]) & (k_arr < K)[:, None]
    ).astype(np.float32)
    maskI2 = np.ascontiguousarray(np.tile(maskD, (1, 2)))

    in_maps = []
    for n in range(N):
        lat8_n = lat8_all[n * T : (n + 1) * T]  # (T, C) fp8
        latT8 = np.ascontiguousarray(lat8_n.reshape(T, 2, 128).transpose(2, 1, 0))
        # stream block per (t, h): 12 positive cols ++ 128 negatives
        phi8 = np.zeros((128, TPAD, 2, FB), dtype=FP8)
        # positives: phi8[p, t, h, j] = lat8_n[1+t+j, h*128+p], t < Tp
        win_idx = 1 + np.arange(Tp)[:, None] + np.arange(PB)[None, :]  # (Tp, PB)
        win = lat8_n[win_idx]  # (Tp, PB, C)
        phi8[:, :Tp, :, :PB] = win.reshape(Tp, PB, 2, 128).transpose(3, 0, 2, 1)
        # negatives: phi8[p, t, h, PB+m] = lat8_all[samps[n,t,m], h*128+p]
        neg = lat8_all[samps[n]]  # (Tp, M, C) fp8
        phi8[:, :Tp, :, PB:] = neg.reshape(Tp, M, 2, 128).transpose(3, 0, 2, 1)
        in_maps.append(
            {
                "phi8": np.ascontiguousarray(phi8.reshape(128, TPAD * 2 * FB)),
                "latT8": latT8,
                "wT8": wT8,
                "pmask": pmask,
                "maskI": maskI2,
            }
        )
    return in_maps


_NC_CACHE = None


def kernel(latent, W, samps):
    global _NC_CACHE
    from concourse import bass_utils

    if _NC_CACHE is None:
        _NC_CACHE = build_bass()
    nc = _NC_CACHE
    in_maps = prep_inputs(latent, W, samps)
    res = bass_utils.run_bass_kernel_spmd(nc, in_maps, core_ids=list(range(N)))
    partial = sum(float(r["out"][0, 0]) for r in res.results)
    import math

    return np.float32(partial / DENOM + SHIFT + 32.0 * math.log(2.0))


# revision 42
# speedup vs baseline: 1.0076x; 1.0076x over previous
"""CPC loss kernel for Trainium2 (8 NeuronCores, SPMD data-parallel over batch N).

Math (per batch element n, handled by core n):
  Az[t]   = W @ latent[n, t]            (K*C = 3072 outputs per position)
  scores[t, k, m] = phi[s_{t,m}] . Az[t, k]   (M=128 negatives per position)
  num[t, k]       = latent[n, 1+t+k] . Az[t, k]
  loss = mean over (n, t<500, k) of log(sum_m exp(scores) + exp(num)) - num

Device strategy per core (fp8 streaming, no on-device gather):
  - The negative-sample gather is a pure data rearrangement with indices known
    at kernel-build time, so the host materializes the per-position rhs stream
    in fp8-e4m3 directly in the [c, cols]-layout the PE needs: for each
    (position t, c-half h) a 140-col block = 12 positive latent cols
    (latent[1+t+j], diagonal j==k extracted later) ++ 128 gathered negatives.
    The device streams it with plain wide DMAs (16 chunks, ~1.15 MB each,
    ~18.4 MB total) -- no SWDGE descriptor generation at all.
  - AzT is computed once via 24 fp8 DoubleRow matmuls (one per kc-tile, both
    c-halves contracted per instruction) into 2-bank PSUM tiles and stored fp8
    in SBUF as azk[c_half, k, h, t] with one contiguous 1 KB/partition copy
    per k (alternating ACT/DVE); score-matmul lhsT slabs are strided column
    views of it.
  - Per position one matmul pair (h-halves accumulating) with lhsT
    azk[:, :, h, t] computes positives and negatives together: [32, 140] into
    the half-megatile's PSUM bank at tile_position (0, 32q).  Per 8-position
    half-megatile (2 banks, 4-deep pipeline) one ACT exp(x-50) (bf16 out) +
    DVE reduce yields sum_m exp(scores-50) and one batched DVE mul+reduce
    extracts num; exp(num-50) is folded into tot once at the end.
  - Final: ln(tot*2^-32) with accum_out row-sum, minus the num row-sum,
    masked partition-sum via 1-col matmul.
Host: loss = sum(partials)/48000 + 50 + 32*ln(2).
"""

import sys, os

_ABL = ""

for _p in ("/opt/trn_rl_repo", "/root/.axon_site/_ro/trn_rl_repo"):
    if _p not in sys.path:
        sys.path.append(_p)

import numpy as np
import ml_dtypes

import concourse.bass as bass
import concourse.bacc as bacc
import concourse.mybir as mybir
from concourse.tile import TileContext, add_dep_helper

BF16 = ml_dtypes.bfloat16
FP8 = ml_dtypes.float8_e4m3

N, T, C, K, M = 8, 512, 256, 12, 128
Tp = T - K  # 500 real positions
TPAD = 512  # padded position count (64 PSUM half-megatiles of 8)
PB = 12  # positive cols per position (k = 0..11)
FB = PB + M  # 140 stream cols per (position, c-half)
SHIFT = 50.0  # fixed logsumexp shift; |scores| << SHIFT + 88 so exp never overflows
DENOM = N * Tp * K  # 48000
NCHUNK = 16  # phi streaming chunks (32 positions each)
CPOS = TPAD // NCHUNK  # positions per chunk


def build_bass():
    nc = bacc.Bacc(
        "TRN2",
        target_bir_lowering=False,
        debug=False,
        enable_asserts=False,
    )
    dt = mybir.dt
    DR = mybir.MatmulPerfMode.DoubleRow

    # phi8[p, t, h, 0:12]   = fp8(latent[n, 1+t+j, h*128 + p]) (positives)
    # phi8[p, t, h, 12:140] = fp8(latent[samps[n,t,m] // T, _ % T, h*128 + p])
    phi8 = nc.dram_tensor("phi8", [128, TPAD * 2 * FB], dt.float8e4, kind="ExternalInput").ap()
    latT8 = nc.dram_tensor("latT8", [128, 2, T], dt.float8e4, kind="ExternalInput").ap()
    wT8 = nc.dram_tensor("wT8", [128, 2 * K, 2, 128], dt.float8e4, kind="ExternalInput").ap()
    pmask = nc.dram_tensor("pmask", [128, 1], dt.float32, kind="ExternalInput").ap()
    maskI = nc.dram_tensor("maskI", [128, 2 * PB], dt.float32, kind="ExternalInput").ap()
    out = nc.dram_tensor("out", [1, 1], dt.float32, kind="ExternalOutput").ap()

    with TileContext(nc) as tc:
        with (
            tc.tile_pool(name="const", bufs=1) as cp,
            tc.tile_pool(name="phi", bufs=1) as pp,
            tc.tile_pool(name="scr", bufs=8) as sp,
            tc.tile_pool(name="acc", bufs=1) as ap_,
        ):
            # --- constant / weight loads -------------------------------------
            latT8_t = cp.tile([128, 2, T], dt.float8e4)
            nc.sync.dma_start(latT8_t[:], latT8[:])
            # per-kc-tile-major W layout, split into two DMAs so the first Az
            # matmuls can start as soon as the first half lands
            wT8_t = cp.tile([128, 2 * K, 2, 128], dt.float8e4)
            nc.sync.dma_start(wT8_t[:, :K], wT8[:, :K])
            nc.sync.dma_start(wT8_t[:, K:], wT8[:, K:])
            negshift = cp.tile([128, 1], dt.float32)
            nc.vector.memset(negshift[:], -SHIFT)
            # preload both ACT table sets (exp, ln) while the DMAs run so the
            # first real exp / final ln don't stall on a ~1.3us table load
            tldca = cp.tile([128, 1], dt.float32)
            tldcb = cp.tile([128, 1], dt.float32)
            nc.vector.memset(tldca[:], 1.0)
            nc.scalar.activation(out=tldcb[:], in_=tldca[:], func=mybir.ActivationFunctionType.Exp)
            nc.scalar.activation(out=tldcb[:], in_=tldca[:], func=mybir.ActivationFunctionType.Ln)

            pmask_t = cp.tile([128, 1], dt.float32)
            nc.sync.dma_start(pmask_t[:], pmask[:])
            maskI_t = cp.tile([128, 2, PB], dt.float32)
            nc.sync.dma_start(maskI_t[:], maskI[:].rearrange("p (s j) -> p s j", j=PB))

            # AzT store: azk[p, k, h, t] = Az[k, h*128+p, t] (fp8), k padded
            # to 32 with zeros so the pad output partitions produce zero
            # scores.  (h, t) innermost so each k's PSUM->SBUF copy is one
            # contiguous 1 KB/partition move; lhsT slabs take strided columns.
            azsb = ap_.tile([128, 32 * 2 * T], dt.float8e4)
            azk = azsb.rearrange("p (k hh t) -> p k hh t", hh=2, k=32)
            # pad k-cols zeroed on DVE: it is idle during the PE warmup and
            # the first Az matmuls, and keeping this fill off the DMA queue
            # preserves the phi chunks' arrival margin over their consumers
            azpad = azk[:, K:32, :, :]
            nc.vector.memzero(azpad[:, : (32 - K) // 2])
            nc.vector.memzero(azpad[:, (32 - K) // 2 :])

            # --- phi stream: 16 chunks of 32 positions (~9 KB/partition each)
            phi_t = pp.tile([128, TPAD * 2 * FB], dt.float8e4)
            phi4 = phi_t.rearrange("p (t hh j) -> p t hh j", hh=2, j=FB)
            for ch in range(NCHUNK):
                c0 = ch * CPOS * 2 * FB
                c1 = (ch + 1) * CPOS * 2 * FB
                nc.sync.dma_start(phi_t[:, c0:c1], phi8[:, c0:c1])

            # tot in bf16: 2-byte src+dst lets the DVE reduce run in 2x_1P
            # packed mode (validated: < 1e-5 impact on the loss)
            tot_all = ap_.tile([128, TPAD // 4], dt.bfloat16)
            num_all = ap_.tile([128, TPAD // 4], dt.float32)

            # --- Az phase: AzT[kc, t] = sum_c' W[kc, c'] latent[n, t, c'] ----
            # One DoubleRow matmul per kc-tile contracts both c'-halves; both
            # h-halves of a k land in one 2-bank PSUM tile so a single
            # contiguous copy (alternating ACT/DVE) moves them to SBUF.
            # The Az and score phases share one PSUM pool (same tile shape) so
            # there is no pool-transition barrier between them.
            wtile = cp.tile([128, 256], dt.bfloat16)
            nc.vector.memset(wtile[:], 0.5)
            with tc.tile_pool(name="ps", bufs=4, space="PSUM") as scps:
                # dummy matmuls ramp the PE p-state out of its cold clock
                # while the weight DMAs are still in flight, so the real Az
                # matmuls don't run 6x slow
                W0 = scps.tile([128, 2, T], dt.float32, name="P")
                for _ in range(12):
                    nc.tensor.matmul(
                        W0[:, 0, 0:128],
                        lhsT=wtile[:, 0:128],
                        rhs=wtile[:, 128:256],
                        start=True,
                        stop=True,
                    )
                for k_ in range(K):
                    pa = scps.tile([128, 2, T], dt.float32, name="P")
                    for h_ in range(2):
                        nc.tensor.matmul(
                            pa[:, h_, :],
                            lhsT=wT8_t[:, 2 * k_ + h_, :, :],
                            rhs=latT8_t[:, :, :],
                            start=True,
                            stop=True,
                            perf_mode=DR,
                        )
                    if k_ % 2 == 0:
                        nc.scalar.copy(out=azk[:, k_, :, :], in_=pa[:, :, :])
                    else:
                        nc.vector.tensor_copy(out=azk[:, k_, :, :], in_=pa[:, :, :])

                # --- score half-megatiles ------------------------------------
                _nmega = TPAD // 8
                for mega in range(_nmega):
                    P = scps.tile([128, 2, 512], dt.float32, name="P")
                    if "nomm" not in _ABL:
                        for s in range(2):  # bank = one 4-position score tile
                            for q in range(4):
                                t = (mega * 2 + s) * 4 + q
                                for h in range(2):
                                    nc.tensor.matmul(
                                        P[32 * q : 32 * q + 32, s, 0:FB],
                                        lhsT=azk[:, :, h, t],
                                        rhs=phi4[:, t, h, :],
                                        start=(h == 0),
                                        stop=(h == 1),
                                        tile_position=(0, 32 * q),
                                    )
                    if "notail" in _ABL:
                        continue
                    # tot[t,k] = sum_m exp(score-50): one exp over both banks
                    E4 = sp.tile([128, 2, M], dt.bfloat16, tag="exp", name="exp_o")
                    nc.scalar.activation(
                        out=E4[:],
                        in_=P[:, :, PB : PB + M],
                        func=mybir.ActivationFunctionType.Exp,
                        bias=negshift[:],
                        scale=1.0,
                    )
                    # num[t,k] -> num_all (diagonal j==k of the pos blocks),
                    # both banks in one batched mul+reduce.  The mul goes on
                    # the DVE queue BEFORE the exp-dependent tot-reduce: the
                    # PSUM buffer is released by {exp, mul}, so this keeps the
                    # release off the exp->reduce semaphore chain.
                    scr = sp.tile([128, 2, PB], dt.float32, tag="ttr", name="ttr_o")
                    nc.vector.tensor_mul(scr[:], P[:, :, 0:PB], maskI_t[:])
                    nc.vector.tensor_reduce(
                        num_all[:, mega * 2 : mega * 2 + 2],
                        scr[:],
                        axis=mybir.AxisListType.X,
                        op=mybir.AluOpType.add,
                    )
                    with nc.allow_low_precision(reason="bf16 tot validated <1e-5"):
                        nc.vector.tensor_reduce(
                            tot_all[:, mega * 2 : mega * 2 + 2],
                            E4[:],
                            axis=mybir.AxisListType.X,
                            op=mybir.AluOpType.add,
                        )

            # --- final reduction --------------------------------------------
            if "nofin" in _ABL:
                dummy = ap_.tile([1, 1], dt.float32)
                nc.vector.memset(dummy[:], 0.0)
                nc.sync.dma_start(out[:], dummy[:])
            else:
                NV = Tp // 4  # 125 valid score tiles
                # row-sum of num (DVE) runs concurrently with exp(num-50) (ACT)
                numsum = ap_.tile([128, 1], dt.float32)
                nc.vector.tensor_reduce(
                    numsum[:],
                    num_all[:, :NV],
                    axis=mybir.AxisListType.X,
                    op=mybir.AluOpType.add,
                )
                # fold in the positive term for all valid tiles at once:
                # tot += exp(num - 50)
                en_t = ap_.tile([128, NV], dt.bfloat16)
                nc.scalar.activation(
                    out=en_t[:],
                    in_=num_all[:, :NV],
                    func=mybir.ActivationFunctionType.Exp,
                    bias=negshift[:],
                    scale=1.0,
                )
                nc.vector.tensor_add(
                    tot_all[:, :NV], tot_all[:, :NV], en_t[:]
                )
                # ln(tot * 2^-32) keeps the ACT-ln input within its 2^64 valid
                # range for extreme scores; +32*ln2 is restored on the host.
                # accum_out row-sums the ln values in the same instruction.
                Lt = ap_.tile([128, NV], dt.float32)
                lnsum = ap_.tile([128, 1], dt.float32)
                nc.scalar.activation(
                    out=Lt[:],
                    in_=tot_all[:, :NV],
                    func=mybir.ActivationFunctionType.Ln,
                    scale=float(2.0**-32),
                    accum_out=lnsum[:],
                )
                rs = ap_.tile([128, 1], dt.float32)
                nc.vector.tensor_sub(rs[:], lnsum[:], numsum[:])
                with tc.tile_pool(name="f_ps", bufs=1, space="PSUM") as fps:
                    psf = fps.tile([1, 1], dt.float32)
                    nc.tensor.matmul(psf[:], lhsT=rs[:], rhs=pmask_t[:])
                    outsb = ap_.tile([1, 1], dt.float32)
                    nc.scalar.copy(out=outsb[:], in_=psf[:])
                    nc.sync.dma_start(out[:], outsb[:])

    nc.compile()
    return nc


def prep_inputs(latent, W, samps):
    """Host-side sharding + layout marshalling. Returns per-core input maps."""
    latent = np.asarray(latent, dtype=np.float32)
    W = np.asarray(W, dtype=np.float32)
    samps = np.asarray(samps).astype(np.int64).reshape(N, Tp, M)

    lat8_all = latent.reshape(N * T, C).astype(FP8)
    # wT8[p, b, h, j] = W[b*128 + j, h*128 + p]
    wT8 = np.ascontiguousarray(
        W.astype(FP8).reshape(2 * K, 128, 2, 128).transpose(3, 0, 2, 1)
    )
    pmask = ((np.arange(128) % 32) < K).astype(np.float32).reshape(128, 1)
    k_arr = np.arange(128) % 32
    maskD = (
        (np.arange(PB)[None, :] == k_arr[:, None]) & (k_arr < K)[:, None]
    ).astype(np.float32)
    maskI2 = np.ascontiguousarray(np.tile(maskD, (1, 2)))

    in_maps = []
    for n in range(N):
        lat8_n = lat8_all[n * T : (n + 1) * T]  # (T, C) fp8
        latT8 = np.ascontiguousarray(lat8_n.reshape(T, 2, 128).transpose(2, 1, 0))
        # stream block per (t, h): 12 positive cols ++ 128 negatives
        phi8 = np.zeros((128, TPAD, 2, FB), dtype=FP8)
        # positives: phi8[p, t, h, j] = lat8_n[1+t+j, h*128+p], t < Tp
        win_idx = 1 + np.arange(Tp)[:, None] + np.arange(PB)[None, :]  # (Tp, PB)
        win = lat8_n[win_idx]  # (Tp, PB, C)
        phi8[:, :Tp, :, :PB] = win.reshape(Tp, PB, 2, 128).transpose(3, 0, 2, 1)
        # negatives: phi8[p, t, h, PB+m] = lat8_all[samps[n,t,m], h*128+p]
        neg = lat8_all[samps[n]]  # (Tp, M, C) fp8
        phi8[:, :Tp, :, PB:] = neg.reshape(Tp, M, 2, 128).transpose(3, 0, 2, 1)
        in_maps.append(
            {
                "phi8": np.ascontiguousarray(phi8.reshape(128, TPAD * 2 * FB)),
                "latT8": latT8,
                "wT8": wT8,
                "pmask": pmask,
                "maskI": maskI2,
            }
        )
    return in_maps


_NC_CACHE = None


def kernel(latent, W, samps):
    global _NC_CACHE
    from concourse import bass_utils

    if _NC_CACHE is None:
        _NC_CACHE = build_bass()
    nc = _NC_CACHE
    in_maps = prep_inputs(latent, W, samps)
    res = bass_utils.run_bass_kernel_spmd(nc, in_maps, core_ids=list(range(N)))
    partial = sum(float(r["out"][0, 0]) for r in res.results)
    import math

    return np.float32(partial / DENOM + SHIFT + 32.0 * math.log(2.0))


# revision 43
# speedup vs baseline: 1.0367x; 1.0289x over previous
"""CPC loss kernel for Trainium2 (8 NeuronCores, SPMD data-parallel over batch N).

Math (per batch element n, handled by core n):
  Az[t]   = W @ latent[n, t]            (K*C = 3072 outputs per position)
  scores[t, k, m] = phi[s_{t,m}] . Az[t, k]   (M=128 negatives per position)
  num[t, k]       = latent[n, 1+t+k] . Az[t, k]
  loss = mean over (n, t<500, k) of log(sum_m exp(scores) + exp(num)) - num

Device strategy per core (fp8 streaming, no on-device gather):
  - The negative-sample gather is a pure data rearrangement with indices known
    at kernel-build time, so the host materializes the per-position rhs stream
    in fp8-e4m3 directly in the [c, cols]-layout the PE needs: for each
    (position t, c-half h) a 140-col block = 12 positive latent cols
    (latent[1+t+j], diagonal j==k extracted later) ++ 128 gathered negatives.
    The device streams it with plain wide DMAs (16 chunks, ~1.15 MB each,
    ~18.4 MB total) -- no SWDGE descriptor generation at all.
  - AzT is computed once via 24 fp8 DoubleRow matmuls (one per kc-tile, both
    c-halves contracted per instruction) into 2-bank PSUM tiles and stored fp8
    in SBUF as azk[c_half, k, h, t] with one contiguous 1 KB/partition copy
    per k (alternating ACT/DVE); score-matmul lhsT slabs are strided column
    views of it.
  - Per position one matmul pair (h-halves accumulating) with lhsT
    azk[:, :, h, t] computes positives and negatives together: [32, 140] into
    the half-megatile's PSUM bank at tile_position (0, 32q).  Per 8-position
    half-megatile (2 banks, 4-deep pipeline) one ACT exp(x-50) (bf16 out) +
    DVE reduce yields sum_m exp(scores-50) and one batched DVE mul+reduce
    extracts num; exp(num-50) is folded into tot once at the end.
  - Final: ln(tot*2^-32) with accum_out row-sum, minus the num row-sum,
    masked partition-sum via 1-col matmul.
Host: loss = sum(partials)/48000 + 50 + 32*ln(2).
"""

import sys, os

_ABL = ""

for _p in ("/opt/trn_rl_repo", "/root/.axon_site/_ro/trn_rl_repo"):
    if _p not in sys.path:
        sys.path.append(_p)

import numpy as np
import ml_dtypes

import concourse.bass as bass
import concourse.bacc as bacc
import concourse.mybir as mybir
from concourse.tile import TileContext, add_dep_helper

BF16 = ml_dtypes.bfloat16
FP8 = ml_dtypes.float8_e4m3

N, T, C, K, M = 8, 512, 256, 12, 128
Tp = T - K  # 500 real positions
TPAD = 512  # padded position count (64 PSUM half-megatiles of 8)
PB = 12  # positive cols per position (k = 0..11)
FB = PB + M  # 140 stream cols per (position, c-half)
SHIFT = 50.0  # fixed logsumexp shift; |scores| << SHIFT + 88 so exp never overflows
DENOM = N * Tp * K  # 48000
NCHUNK = 16  # phi streaming chunks (32 positions each)
CPOS = TPAD // NCHUNK  # positions per chunk


def build_bass():
    nc = bacc.Bacc(
        "TRN2",
        target_bir_lowering=False,
        debug=False,
        enable_asserts=False,
    )
    dt = mybir.dt
    DR = mybir.MatmulPerfMode.DoubleRow

    # phi8[p, t, h, 0:12]   = fp8(latent[n, 1+t+j, h*128 + p]) (positives)
    # phi8[p, t, h, 12:140] = fp8(latent[samps[n,t,m] // T, _ % T, h*128 + p])
    phi8 = nc.dram_tensor("phi8", [128, TPAD * 2 * FB], dt.float8e4, kind="ExternalInput").ap()
    latT8 = nc.dram_tensor("latT8", [128, 2, T], dt.float8e4, kind="ExternalInput").ap()
    wT8 = nc.dram_tensor("wT8", [128, 2 * K, 2, 128], dt.float8e4, kind="ExternalInput").ap()
    pmask = nc.dram_tensor("pmask", [128, 1], dt.float32, kind="ExternalInput").ap()
    maskI = nc.dram_tensor("maskI", [128, 2 * PB], dt.float32, kind="ExternalInput").ap()
    out = nc.dram_tensor("out", [1, 1], dt.float32, kind="ExternalOutput").ap()

    with TileContext(nc) as tc:
        with (
            tc.tile_pool(name="const", bufs=1) as cp,
            tc.tile_pool(name="phi", bufs=1) as pp,
            tc.tile_pool(name="scr", bufs=8) as sp,
            tc.tile_pool(name="acc", bufs=1) as ap_,
        ):
            # --- constant / weight loads -------------------------------------
            latT8_t = cp.tile([128, 2, T], dt.float8e4)
            nc.sync.dma_start(latT8_t[:], latT8[:])
            # per-kc-tile-major W layout, split into two DMAs so the first Az
            # matmuls can start as soon as the first half lands
            wT8_t = cp.tile([128, 2 * K, 2, 128], dt.float8e4)
            nc.sync.dma_start(wT8_t[:, :K], wT8[:, :K])
            nc.sync.dma_start(wT8_t[:, K:], wT8[:, K:])
            negshift = cp.tile([128, 1], dt.float32)
            nc.vector.memset(negshift[:], -SHIFT)
            # preload both ACT table sets (exp, ln) while the DMAs run so the
            # first real exp / final ln don't stall on a ~1.3us table load
            tldca = cp.tile([128, 1], dt.float32)
            tldcb = cp.tile([128, 1], dt.float32)
            nc.vector.memset(tldca[:], 1.0)
            nc.scalar.activation(out=tldcb[:], in_=tldca[:], func=mybir.ActivationFunctionType.Exp)
            nc.scalar.activation(out=tldcb[:], in_=tldca[:], func=mybir.ActivationFunctionType.Ln)

            pmask_t = cp.tile([128, 1], dt.float32)
            nc.sync.dma_start(pmask_t[:], pmask[:])
            maskI_t = cp.tile([128, 2, PB], dt.float32)
            nc.sync.dma_start(maskI_t[:], maskI[:].rearrange("p (s j) -> p s j", j=PB))

            # AzT store: azk[p, k, h, t] = Az[k, h*128+p, t] (fp8), k padded
            # to 32 with zeros so the pad output partitions produce zero
            # scores.  (h, t) innermost so each k's PSUM->SBUF copy is one
            # contiguous 1 KB/partition move; lhsT slabs take strided columns.
            azsb = ap_.tile([128, 32 * 2 * T], dt.float8e4)
            azk = azsb.rearrange("p (k hh t) -> p k hh t", hh=2, k=32)
            # pad k-cols zeroed on DVE: it is idle during the PE warmup and
            # the first Az matmuls, and keeping this fill off the DMA queue
            # preserves the phi chunks' arrival margin over their consumers
            azpad = azk[:, K:32, :, :]
            nc.vector.memzero(azpad[:, : (32 - K) // 2])
            nc.vector.memzero(azpad[:, (32 - K) // 2 :])

            # --- phi stream: 16 chunks of 32 positions (~9 KB/partition each)
            phi_t = pp.tile([128, TPAD * 2 * FB], dt.float8e4)
            phi4 = phi_t.rearrange("p (t hh j) -> p t hh j", hh=2, j=FB)
            for ch in range(NCHUNK):
                c0 = ch * CPOS * 2 * FB
                c1 = (ch + 1) * CPOS * 2 * FB
                nc.sync.dma_start(phi_t[:, c0:c1], phi8[:, c0:c1])

            # tot in bf16: 2-byte src+dst lets the DVE reduce run in 2x_1P
            # packed mode (validated: < 1e-5 impact on the loss)
            tot_all = ap_.tile([128, TPAD // 4], dt.bfloat16)
            num_all = ap_.tile([128, TPAD // 4], dt.float32)

            # --- Az phase: AzT[kc, t] = sum_c' W[kc, c'] latent[n, t, c'] ----
            # One DoubleRow matmul per kc-tile contracts both c'-halves; both
            # h-halves of a k land in one 2-bank PSUM tile so a single
            # contiguous copy (alternating ACT/DVE) moves them to SBUF.
            # The Az and score phases share one PSUM pool (same tile shape) so
            # there is no pool-transition barrier between them.
            wtile = cp.tile([128, 256], dt.bfloat16)
            nc.vector.memset(wtile[:], 0.5)
            with tc.tile_pool(name="ps", bufs=4, space="PSUM") as scps:
                # dummy matmuls ramp the PE p-state out of its cold clock
                # while the weight DMAs are still in flight, so the real Az
                # matmuls don't run 6x slow
                W0 = scps.tile([128, 2, T], dt.float32, name="P")
                for _ in range(12):
                    nc.tensor.matmul(
                        W0[:, 0, 0:128],
                        lhsT=wtile[:, 0:128],
                        rhs=wtile[:, 128:256],
                        start=True,
                        stop=True,
                    )
                for k_ in range(K):
                    pa = scps.tile([128, 2, T], dt.float32, name="P")
                    for h_ in range(2):
                        nc.tensor.matmul(
                            pa[:, h_, :],
                            lhsT=wT8_t[:, 2 * k_ + h_, :, :],
                            rhs=latT8_t[:, :, :],
                            start=True,
                            stop=True,
                            perf_mode=DR,
                        )
                    if k_ % 2 == 0:
                        nc.scalar.copy(out=azk[:, k_, :, :], in_=pa[:, :, :])
                    else:
                        nc.vector.tensor_copy(out=azk[:, k_, :, :], in_=pa[:, :, :])

                # --- score half-megatiles ------------------------------------
                # the last half-megatile (positions 504-511) is pure padding:
                # its tile columns 126-127 are never read by the final
                # reduction (NV=125), so it is skipped entirely
                _nmega = TPAD // 8 - 1
                for mega in range(_nmega):
                    P = scps.tile([128, 2, 512], dt.float32, name="P")
                    if "nomm" not in _ABL:
                        for s in range(2):  # bank = one 4-position score tile
                            for q in range(4):
                                t = (mega * 2 + s) * 4 + q
                                for h in range(2):
                                    nc.tensor.matmul(
                                        P[32 * q : 32 * q + 32, s, 0:FB],
                                        lhsT=azk[:, :, h, t],
                                        rhs=phi4[:, t, h, :],
                                        start=(h == 0),
                                        stop=(h == 1),
                                        tile_position=(0, 32 * q),
                                    )
                    if "notail" in _ABL:
                        continue
                    # tot[t,k] = sum_m exp(score-50): one exp over both banks
                    E4 = sp.tile([128, 2, M], dt.bfloat16, tag="exp", name="exp_o")
                    nc.scalar.activation(
                        out=E4[:],
                        in_=P[:, :, PB : PB + M],
                        func=mybir.ActivationFunctionType.Exp,
                        bias=negshift[:],
                        scale=1.0,
                    )
                    # num[t,k] -> num_all (diagonal j==k of the pos blocks),
                    # both banks in one batched mul+reduce.  The mul goes on
                    # the DVE queue BEFORE the exp-dependent tot-reduce: the
                    # PSUM buffer is released by {exp, mul}, so this keeps the
                    # release off the exp->reduce semaphore chain.
                    scr = sp.tile([128, 2, PB], dt.float32, tag="ttr", name="ttr_o")
                    nc.vector.tensor_mul(scr[:], P[:, :, 0:PB], maskI_t[:])
                    nc.vector.tensor_reduce(
                        num_all[:, mega * 2 : mega * 2 + 2],
                        scr[:],
                        axis=mybir.AxisListType.X,
                        op=mybir.AluOpType.add,
                    )
                    with nc.allow_low_precision(reason="bf16 tot validated <1e-5"):
                        nc.vector.tensor_reduce(
                            tot_all[:, mega * 2 : mega * 2 + 2],
                            E4[:],
                            axis=mybir.AxisListType.X,
                            op=mybir.AluOpType.add,
                        )

            # --- final reduction --------------------------------------------
            if "nofin" in _ABL:
                dummy = ap_.tile([1, 1], dt.float32)
                nc.vector.memset(dummy[:], 0.0)
                nc.sync.dma_start(out[:], dummy[:])
            else:
                NV = Tp // 4  # 125 valid score tiles
                # row-sum of num (DVE) runs concurrently with exp(num-50) (ACT)
                numsum = ap_.tile([128, 1], dt.float32)
                nc.vector.tensor_reduce(
                    numsum[:],
                    num_all[:, :NV],
                    axis=mybir.AxisListType.X,
                    op=mybir.AluOpType.add,
                )
                # fold in the positive term for all valid tiles at once:
                # tot += exp(num - 50)
                en_t = ap_.tile([128, NV], dt.bfloat16)
                nc.scalar.activation(
                    out=en_t[:],
                    in_=num_all[:, :NV],
                    func=mybir.ActivationFunctionType.Exp,
                    bias=negshift[:],
                    scale=1.0,
                )
                nc.vector.tensor_add(
                    tot_all[:, :NV], tot_all[:, :NV], en_t[:]
                )
                # ln(tot * 2^-32) keeps the ACT-ln input within its 2^64 valid
                # range for extreme scores; +32*ln2 is restored on the host.
                # accum_out row-sums the ln values in the same instruction.
                Lt = ap_.tile([128, NV], dt.float32)
                lnsum = ap_.tile([128, 1], dt.float32)
                nc.scalar.activation(
                    out=Lt[:],
                    in_=tot_all[:, :NV],
                    func=mybir.ActivationFunctionType.Ln,
                    scale=float(2.0**-32),
                    accum_out=lnsum[:],
                )
                rs = ap_.tile([128, 1], dt.float32)
                nc.vector.tensor_sub(rs[:], lnsum[:], numsum[:])
                with tc.tile_pool(name="f_ps", bufs=1, space="PSUM") as fps:
                    psf = fps.tile([1, 1], dt.float32)
                    nc.tensor.matmul(psf[:], lhsT=rs[:], rhs=pmask_t[:])
                    outsb = ap_.tile([1, 1], dt.float32)
                    nc.scalar.copy(out=outsb[:], in_=psf[:])
                    nc.sync.dma_start(out[:], outsb[:])

    nc.compile()
    return nc


def prep_inputs(latent, W, samps):
    """Host-side sharding + layout marshalling. Returns per-core input maps."""
    latent = np.asarray(latent, dtype=np.float32)
    W = np.asarray(W, dtype=np.float32)
    samps = np.asarray(samps).astype(np.int64).reshape(N, Tp, M)

    lat8_all = latent.reshape(N * T, C).astype(FP8)
    # wT8[p, b, h, j] = W[b*128 + j, h*128 + p]
    wT8 = np.ascontiguousarray(
        W.astype(FP8).reshape(2 * K, 128, 2, 128).transpose(3, 0, 2, 1)
    )
    pmask = ((np.arange(128) % 32) < K).astype(np.float32).reshape(128, 1)
    k_arr = np.arange(128) % 32
    maskD = (
        (np.arange(PB)[None, :] == k_arr[:, None]) & (k_arr < K)[:, None]
    ).astype(np.float32)
    maskI2 = np.ascontiguousarray(np.tile(maskD, (1, 2)))

    in_maps = []
    for n in range(N):
        lat8_n = lat8_all[n * T : (n + 1) * T]  # (T, C) fp8
        latT8 = np.ascontiguousarray(lat8_n.reshape(T, 2, 128).transpose(2, 1, 0))
        # stream block per (t, h): 12 positive cols ++ 128 negatives
        phi8 = np.zeros((128, TPAD, 2, FB), dtype=FP8)
        # positives: phi8[p, t, h, j] = lat8_n[1+t+j, h*128+p], t < Tp
        win_idx = 1 + np.arange(Tp)[:, None] + np.arange(PB)[None, :]  # (Tp, PB)
        win = lat8_n[win_idx]  # (Tp, PB, C)
        phi8[:, :Tp, :, :PB] = win.reshape(Tp, PB, 2, 128).transpose(3, 0, 2, 1)
        # negatives: phi8[p, t, h, PB+m] = lat8_all[samps[n,t,m], h*128+p]
        neg = lat8_all[samps[n]]  # (Tp, M, C) fp8
        phi8[:, :Tp, :, PB:] = neg.reshape(Tp, M, 2, 128).transpose(3, 0, 2, 1)
        in_maps.append(
            {
                "phi8": np.ascontiguousarray(phi8.reshape(128, TPAD * 2 * FB)),
                "latT8": latT8,
                "wT8": wT8,
                "pmask": pmask,
                "maskI": maskI2,
            }
        )
    return in_maps


_NC_CACHE = None


def kernel(latent, W, samps):
    global _NC_CACHE
    from concourse import bass_utils

    if _NC_CACHE is None:
        _NC_CACHE = build_bass()
    nc = _NC_CACHE
    in_maps = prep_inputs(latent, W, samps)
    res = bass_utils.run_bass_kernel_spmd(nc, in_maps, core_ids=list(range(N)))
    partial = sum(float(r["out"][0, 0]) for r in res.results)
    import math

    return np.float32(partial / DENOM + SHIFT + 32.0 * math.log(2.0))
